# revision 54
# baseline (speedup 1.0000x reference)
"""AffinityFC Trainium2 kernel (Bass/Tile, 8 NeuronCores, data-parallel over B).

Math per batch b (one NeuronCore per batch):
    px = X[b] @ W1x.T          (Nx=128, hd=1024)
    py = Y[b] @ W1y.T          (Ny=128, hd=1024)
    out[n, m] = W2 . relu(px[n, :] + py[m, :] + b1) + b2

Key reformulation: with s = px + b1,
    relu(py + s) = max(py, -s) + s
so for "max-form" rows the device computes u = max(py, -s) (one DVE
tensor_tensor max per element) and reduces Sum_h W2[h]*u with TensorE;
the missing Sum_h W2[h]*s[n,h] term is a per-chunk rank-1 correction
added on the host.  "relu-form" rows (ScalarE relu(py + s_n), bias per
partition) need no correction for their chunks.

Schedule (v20, ~78.8us fast-clock):
  - PSUM: obanks 0..5 allocated BEFORE the layer-1 pool (disjoint stack
    space, so main matmuls for those banks start immediately); obanks
    6,7 allocated in a sibling pool after layer-1's pool closes and
    reuse its 2 banks.  Bank map: nb 0..23 -> bank nb%6, nb 24..27 ->
    bank 7 (w3 DVE tile + ACT nb 27), nb 28..31 -> bank 6 (ACT rows).
  - DVE does only max ops (one w27 per middle chunk, w28 in chunk 0 —
    the DVE is the saturated engine, zero-gap back-to-back); ScalarE
    does the relu-form rows (relu ops with bias=s[:,n] per partition;
    rows 28-31 in chunks 0..6 and row 27 in chunks 1..6) plus all
    layer-1 evacs (negs/s/pyr) at high priority.
  - input DMAs: chunk-0-critical data merged into ONE DMA per HWDGE
    queue (crx=[xt|w1x_c0] on scalar, cry=[yt|w1y_c0] on sync, y first
    since pyr gates the first MAX), consts via SWDGE, bulk W1 behind.
  - last chunk: bank-grouped DVE ops (w4, last bank split w2+w2)
    feeding a per-bank matmul -> ScalarE evac -> out-DMA pipeline.
  Known hazard: the DVE/PE clocks vary between runs (0.96 vs 0.80 GHz
  observed); compare designs only at equal clock (MAX w24 = 6.47us
  fast, 7.87us slow).
"""

import numpy as np
import ml_dtypes

import concourse.mybir as mybir
import concourse.tile as tile
from concourse import bacc
from concourse.bass import ts
from concourse.bass_utils import run_bass_kernel_spmd

B, NX, NY, D, HD = 8, 128, 128, 512, 1024
NCORES = 8
NCH = HD // 128      # 8 h-chunks
KT = D // 128        # 4 k-tiles for the layer-1 contraction
NBLK = NX // 4       # 32 n-blocks of 4 rows each
F32 = mybir.dt.float32
BF16 = mybir.dt.bfloat16

ACT_NBS = (27, 28, 29, 30, 31)  # relu-form rows (ScalarE) in middle chunks


def relu_form(c, nb):
    # which (chunk, n-block) pairs are produced relu-form on ScalarE:
    # rows 28..31 in chunks 0..6 (chunk 0's relu ops run right after the
    # layer-1 evacs drain, taking them off the saturated DVE), row 27 in
    # the middle chunks only.
    if nb >= 28:
        return c < NCH - 1
    if nb == 27:
        return 1 <= c <= NCH - 2
    return False


def bankmap(nb):  # nb -> (bank, jc)
    if nb < 24:
        return nb % 6, nb // 6
    if nb < 28:
        return 7, nb - 24
    return 6, nb - 28


def _build_nc(do_compile=True):
    nc = bacc.Bacc(
        "TRN2", target_bir_lowering=False, debug=False, num_devices=NCORES
    )

    # flat SBUF images: dram[p, col] == sbuf[p, col]
    # crx/cry bundle the chunk-0-critical data into ONE DMA each:
    #   crx = [xt image (KT*NX) | w1x chunk-0 slab (KT*128)]
    CW0 = KT * 128
    crx = nc.dram_tensor("crx", [128, KT * NX + CW0], BF16, kind="ExternalInput")
    cry = nc.dram_tensor("cry", [128, KT * NY + CW0], BF16, kind="ExternalInput")
    w1xt = nc.dram_tensor("w1xt", [128, (NCH - 1) * CW0], BF16, kind="ExternalInput")
    w1yt = nc.dram_tensor("w1yt", [128, (NCH - 1) * CW0], BF16, kind="ExternalInput")
    b1c = nc.dram_tensor("b1c", [128, 2 * NCH], F32, kind="ExternalInput")
    w2c = nc.dram_tensor("w2c", [128, NCH * 32], BF16, kind="ExternalInput")
    out = nc.dram_tensor("out", [1, NBLK * 512], F32, kind="ExternalOutput")

    with tile.TileContext(nc) as tc:
        with (
            tc.tile_pool(name="const", bufs=1) as cp,
            tc.tile_pool(name="tprod", bufs=9) as tp,
        ):
            crx_sb = cp.tile([128, KT * NX + CW0], BF16)
            cry_sb = cp.tile([128, KT * NY + CW0], BF16)
            xt_sb = crx_sb[:, : KT * NX]
            yt_sb = cry_sb[:, : KT * NY]
            W1GRP = ((1, 2), (2, 4), (4, 6), (6, NCH))
            w1x_g = [
                cp.tile([128, (hi - lo) * KT * 128], BF16, name=f"w1x{lo}")
                for lo, hi in W1GRP
            ]
            w1y_g = [
                cp.tile([128, (hi - lo) * KT * 128], BF16, name=f"w1y{lo}")
                for lo, hi in W1GRP
            ]

            def w1slab(g, c, k):  # lhsT slab for (chunk c, k-tile)
                if c == 0:
                    cr = crx_sb if g is w1x_g else cry_sb
                    off = KT * NX + k * 128
                    return cr[:, off : off + 128]
                for (lo, hi), tile_ in zip(W1GRP, g):
                    if lo <= c < hi:
                        off = ((c - lo) * KT + k) * 128
                        return tile_[:, off : off + 128]
                raise AssertionError
            b1_sb = cp.tile([128, 2 * NCH], F32)    # [+b1 | -b1] chunk columns
            w2_sb = cp.tile([128, NCH * 32], BF16)
            negs_sb = cp.tile([128, HD], BF16)
            s_sb = cp.tile([128, HD], F32)
            pyr_sb = cp.tile([128, NCH * 512], BF16)  # py_rep4 per chunk
            out_sc = cp.tile([128, 8 * 512], F32)

            CW = KT * 128  # image columns per chunk
            # critical pieces on the 2 HWDGE queues, bulk W1 on SWDGE;
            # tiny consts lead (small-DMA completion latency ~1.3us)
            # one merged critical DMA per HWDGE queue; consts on SWDGE;
            # bulk W1 split behind the critical pieces (w1xt/w1yt carry
            # chunks 1..7, so group (lo,hi) maps to cols (lo-1)*CW0..)
            nc.sync.dma_start(out=cry_sb[:, :], in_=cry[:, :])
            nc.scalar.dma_start(out=crx_sb[:, :], in_=crx[:, :])
            # consts ride the HWDGE queues right behind the critical
            # bundles (needed ~1.5us after they land); SWDGE never used
            nc.sync.dma_start(out=b1_sb[:, :], in_=b1c[:, :])
            nc.scalar.dma_start(out=w2_sb[:, :], in_=w2c[:, :])
            for gi, (lo, hi) in enumerate(W1GRP):
                a, b = (lo - 1) * CW0, (hi - 1) * CW0
                nc.scalar.dma_start(out=w1x_g[gi][:, :], in_=w1xt[:, a:b])
                nc.sync.dma_start(out=w1y_g[gi][:, :], in_=w1yt[:, a:b])

            with tc.tile_pool(name="mpsA", bufs=1, space="PSUM") as mpsA:
                obanks = [None] * 8
                for i in range(6):
                    obanks[i] = mpsA.tile([128, 512], F32, name=f"ob{i}", tag=f"ob{i}")

                # ---- layer 1 per h-chunk; all evacs on ScalarE (hi-pri):
                #   negs = -(px+b1) bf16, s = px+b1 f32, pyr = py rep4 bf16
                with tc.tile_pool(name="l1ps", bufs=2, space="PSUM") as l1ps:
                    for c in range(NCH):
                        pyp = l1ps.tile([128, NY], F32, tag="l1")
                        for k in range(KT):
                            nc.tensor.matmul(
                                pyp[:, :],
                                w1slab(w1y_g, c, k),
                                yt_sb[:, ts(k, NY)],
                                start=(k == 0),
                                stop=(k == KT - 1),
                            )
                        with tc.high_priority():
                            nc.scalar.activation(
                                out=pyr_sb[:, ts(c, 512)].rearrange(
                                    "p (m j) -> p m j", j=4
                                ),
                                in_=pyp[:, :].unsqueeze(2).broadcast_to(
                                    (128, 128, 4)
                                ),
                                func=mybir.ActivationFunctionType.Copy,
                            )
                        pxp = l1ps.tile([128, NX], F32, tag="l1")
                        for k in range(KT):
                            nc.tensor.matmul(
                                pxp[:, :],
                                w1slab(w1x_g, c, k),
                                xt_sb[:, ts(k, NX)],
                                start=(k == 0),
                                stop=(k == KT - 1),
                            )
                        if c == 0:
                            # DVE is idle during the ramp and the ACT chain
                            # (pyr then negs) gates the first MAX otherwise
                            with tc.high_priority():
                                nc.vector.tensor_scalar(
                                    out=negs_sb[:, ts(c, 128)],
                                    in0=pxp[:, :],
                                    scalar1=b1_sb[:, c : c + 1],
                                    scalar2=-1.0,
                                    op0=mybir.AluOpType.add,
                                    op1=mybir.AluOpType.mult,
                                )
                        else:
                            with tc.high_priority():
                                nc.scalar.activation(
                                    out=negs_sb[:, ts(c, 128)],
                                    in_=pxp[:, :],
                                    func=mybir.ActivationFunctionType.Identity,
                                    bias=b1_sb[:, NCH + c : NCH + c + 1],
                                    scale=-1.0,
                                )
                        if c != NCH - 1:
                            with tc.high_priority():
                                nc.scalar.activation(
                                    out=s_sb[:, ts(c, 128)],
                                    in_=pxp[:, :],
                                    func=mybir.ActivationFunctionType.Identity,
                                    bias=b1_sb[:, c : c + 1],
                                    scale=1.0,
                                )

                # banks 6,7 reuse layer-1's psum space (deps via allocator)
                mpsB_cm = tc.tile_pool(name="mpsB", bufs=1, space="PSUM")
                mpsB = mpsB_cm.__enter__()
                obanks[6] = mpsB.tile([128, 512], F32, name="ob6", tag="ob6")
                obanks[7] = mpsB.tile([128, 512], F32, name="ob7", tag="ob7")

                # ---- main loop, c-outer; slivers accumulate across chunks
                for c in range(NCH):
                    last = c == NCH - 1
                    pyr_c = pyr_sb[:, ts(c, 512)]
                    pyr3 = pyr_c.rearrange("p (m j) -> p m j", j=4)
                    tslice = {}  # nb -> (tile, column offset index)

                    def dve_max(t, w, in1, prio):
                        in0 = pyr3.unsqueeze(1).broadcast_to((128, w, 128, 4))
                        out_ap = t[:, :].rearrange(
                            "p (nbs m j) -> p nbs m j", nbs=w, m=128
                        )
                        if prio:
                            with tc.high_priority():
                                nc.vector.tensor_tensor(
                                    out=out_ap, in0=in0, in1=in1,
                                    op=mybir.AluOpType.max,
                                )
                        else:
                            nc.vector.tensor_tensor(
                                out=out_ap, in0=in0, in1=in1,
                                op=mybir.AluOpType.max,
                            )

                    def negs_in1(cols, w):
                        # cols: list-slice of negs columns [p, w, 4] -> bcast m
                        return cols.unsqueeze(2).broadcast_to((128, w, 128, 4))

                    if last:
                        # bank-grouped w4 ops so the final bank-major
                        # matmul+evac+DMA pipeline starts per-bank
                        nrr = negs_sb[:, ts(c, 128)].rearrange(
                            "p (nb j) -> p nb j", j=4
                        )
                        for bk in range(8):
                            nbs_list = [nb for nb in range(NBLK) if bankmap(nb)[0] == bk]
                            lo = nbs_list[0]
                            step = nbs_list[1] - nbs_list[0]
                            if bk < 7:
                                t = tp.tile(
                                    [128, 4 * 512], BF16, name=f"tb{bk}",
                                    tag="t4",
                                )
                                in1 = negs_in1(
                                    nrr[:, lo : lo + 3 * step + 1 : step, :], 4
                                )
                                dve_max(t, 4, in1, False)
                                for i, nb in enumerate(nbs_list):
                                    tslice[nb] = (t, i)
                            else:
                                for half in range(2):
                                    t = tp.tile(
                                        [128, 2 * 512], BF16,
                                        name=f"tb{bk}_{half}", tag="t2", bufs=2,
                                    )
                                    sub = nbs_list[2 * half : 2 * half + 2]
                                    in1 = negs_in1(
                                        nrr[:, sub[0] : sub[1] + 1 : step, :], 2
                                    )
                                    dve_max(t, 2, in1, False)
                                    for i, nb in enumerate(sub):
                                        tslice[nb] = (t, i)
                    elif c == 0:
                        t = tp.tile([128, 28 * 512], BF16, name="t0",
                                    tag="t", bufs=4)
                        in1 = negs_in1(
                            negs_sb[:, c * 128 : c * 128 + 112]
                            .rearrange("p (nbs j) -> p nbs j", j=4), 28
                        )
                        dve_max(t, 28, in1, True)
                        for nbs in range(28):
                            tslice[nbs] = (t, nbs)
                        for nb in (28, 29, 30, 31):
                            ta = tp.tile(
                                [128, 512], BF16, name=f"ta{c}_{nb}", tag="ta",
                                bufs=6,
                            )
                            for j in range(4):
                                n = nb * 4 + j
                                nc.scalar.activation(
                                    out=ta[:, :].rearrange(
                                        "p (m j) -> p m j", j=4
                                    )[:, :, j],
                                    in_=pyr3[:, :, j],
                                    func=mybir.ActivationFunctionType.Relu,
                                    bias=s_sb[:, c * 128 + n : c * 128 + n + 1],
                                    scale=1.0,
                                )
                            tslice[nb] = (ta, 0)
                    else:
                        t = tp.tile([128, 27 * 512], BF16, name=f"t{c}",
                                    tag="t", bufs=4)
                        in1 = negs_in1(
                            negs_sb[:, c * 128 : c * 128 + 108]
                            .rearrange("p (nbs j) -> p nbs j", j=4), 27
                        )
                        dve_max(t, 27, in1, False)
                        for nbs in range(27):
                            tslice[nbs] = (t, nbs)
                        # ACT share (relu-form rows, bank 6)
                        for nb in ACT_NBS:
                            ta = tp.tile(
                                [128, 512], BF16, name=f"ta{c}_{nb}", tag="ta", bufs=6
                            )
                            for j in range(4):
                                n = nb * 4 + j
                                nc.scalar.activation(
                                    out=ta[:, :].rearrange("p (m j) -> p m j", j=4)[
                                        :, :, j
                                    ],
                                    in_=pyr3[:, :, j],
                                    func=mybir.ActivationFunctionType.Relu,
                                    bias=s_sb[:, c * 128 + n : c * 128 + n + 1],
                                    scale=1.0,
                                )
                            tslice[nb] = (ta, 0)

                    if not last:
                        for nb in range(NBLK):
                            bk, jc = bankmap(nb)
                            t, nbs = tslice[nb]
                            nc.tensor.matmul(
                                obanks[bk][32 * jc : 32 * jc + 32, :],
                                w2_sb[:, ts(c, 32)],
                                t[:, ts(nbs, 512)],
                                start=(c == 0),
                                stop=False,
                                tile_position=(0, 32 * jc),
                                skip_group_check=True,
                            )
                    else:
                        # bank-major: 4 slivers -> ScalarE evac -> out-DMA
                        for bk in range(8):
                            for nb in range(NBLK):
                                b2_, jc = bankmap(nb)
                                if b2_ != bk:
                                    continue
                                t, nbs = tslice[nb]
                                nc.tensor.matmul(
                                    obanks[bk][32 * jc : 32 * jc + 32, :],
                                    w2_sb[:, ts(c, 32)],
                                    t[:, ts(nbs, 512)],
                                    start=False,
                                    stop=True,
                                    tile_position=(0, 32 * jc),
                                    skip_group_check=True,
                                )
                            nc.scalar.copy(
                                out=out_sc[:, ts(bk, 512)], in_=obanks[bk][:, :]
                            )
                            # raw layout: raw[nb*512 + m*4 + j] for the 4 nb
                            # of this bank (jc = 0..3 at partitions 0/32/64/96)
                            nbs_list = [nb for nb in range(NBLK) if bankmap(nb)[0] == bk]
                            lo = nbs_list[0]
                            step = nbs_list[1] - nbs_list[0]
                            dst = out[:, :].rearrange(
                                "o (nb q) -> (o nb) q", nb=NBLK
                            )[lo : lo + 3 * step + 1 : step, :]
                            src = out_sc[0:128:32, ts(bk, 512)]
                            (nc.sync, nc.scalar)[bk % 2].dma_start(out=dst, in_=src)
                mpsB_cm.__exit__(None, None, None)

    if do_compile:
        nc.compile()
    return nc


_NC_CACHE = None


def _get_nc():
    global _NC_CACHE
    if _NC_CACHE is None:
        _NC_CACHE = _build_nc()
    return _NC_CACHE


def prepare_in_maps(X, Y, W1, b1, W2):
    X = np.asarray(X, dtype=np.float32)
    Y = np.asarray(Y, dtype=np.float32)
    W1 = np.asarray(W1, dtype=np.float32)
    b1 = np.asarray(b1, dtype=np.float32)
    W2 = np.asarray(W2, dtype=np.float32)

    bf = ml_dtypes.bfloat16

    def w1_img(Wh):  # (HD, D) -> flat sbuf image (128, KT*HD)
        return np.ascontiguousarray(
            Wh.reshape(NCH, 128, KT, 128).transpose(3, 0, 2, 1).reshape(128, -1)
        ).astype(bf)

    def xy_img(Xb):  # (N, D) -> flat sbuf image (128, KT*N)
        return np.ascontiguousarray(
            Xb.T.reshape(KT, 128, -1).transpose(1, 0, 2).reshape(128, -1)
        ).astype(bf)

    w1xi = w1_img(W1[:, :D])
    w1yi = w1_img(W1[:, D:])
    CW0 = KT * 128
    b1m = b1.reshape(NCH, 128).T                      # (128, NCH)
    b1cm = np.ascontiguousarray(np.hstack([b1m, -b1m]))  # [+b1 | -b1] f32
    w2cm = np.ascontiguousarray(
        np.repeat(W2.reshape(NCH, 128).T[:, :, None], 32, axis=2).reshape(128, -1)
    ).astype(bf)

    in_maps = []
    for b in range(B):
        in_maps.append(
            {
                "crx": np.ascontiguousarray(
                    np.hstack([xy_img(X[b]), w1xi[:, :CW0]])
                ),
                "cry": np.ascontiguousarray(
                    np.hstack([xy_img(Y[b]), w1yi[:, :CW0]])
                ),
                "w1xt": np.ascontiguousarray(w1xi[:, CW0:]),
                "w1yt": np.ascontiguousarray(w1yi[:, CW0:]),
                "b1c": b1cm,
                "w2c": w2cm,
            }
        )
    return in_maps


def postprocess(raw_outs, X, W1, b1, W2, b2):
    """raw[nb*512 + m*4 + j] = device sum for out row 4nb+j, col m.
    Add the per-(row, chunk-set) max-form correction gamma, then b2."""
    X = np.asarray(X, dtype=np.float32)
    W1 = np.asarray(W1, dtype=np.float32)
    b1 = np.asarray(b1, dtype=np.float32)
    W2 = np.asarray(W2, dtype=np.float32)
    b2 = np.asarray(b2, dtype=np.float32)

    # per-chunk rank-1 pieces: gam_c[b, n] = X[b, n]·v_c + g_c
    Vc = np.stack([
        W2[0, c * 128 : (c + 1) * 128] @ W1[c * 128 : (c + 1) * 128, :D]
        for c in range(NCH)
    ])  # (NCH, D)
    gc = np.array([
        W2[0, c * 128 : (c + 1) * 128] @ b1[c * 128 : (c + 1) * 128]
        for c in range(NCH)
    ])
    maxform = np.ones((NBLK, NCH), dtype=np.float32)
    for nb in range(NBLK):
        for c in range(NCH):
            if relu_form(c, nb):
                maxform[nb, c] = 0.0

    out = np.empty((B, NX, NY), dtype=np.float32)
    for b in range(B):
        r = raw_outs[b].reshape(NBLK, 128, 4)     # (nb, m, j)
        o = r.transpose(0, 2, 1).reshape(NX, NY)  # (4nb+j, m)
        A = X[b] @ Vc.T + gc                      # (NX, NCH)
        gamma = (A.reshape(NBLK, 4, NCH) * maxform[:, None, :]).sum(-1)
        out[b] = o + gamma.reshape(NX)[:, None] + b2[0]
    return out


def kernel(X, Y, W1, b1, W2, b2):
    in_maps = prepare_in_maps(X, Y, W1, b1, W2)
    nc = _get_nc()
    res = run_bass_kernel_spmd(nc, in_maps, core_ids=list(range(NCORES)))
    raw = [res.results[b]["out"].reshape(-1) for b in range(B)]
    return postprocess(raw, X, W1, b1, W2, b2)


if __name__ == "__main__":
    rng = np.random.default_rng(0)
    ins = {
        "X": rng.standard_normal((B, NX, D), dtype=np.float32),
        "Y": rng.standard_normal((B, NY, D), dtype=np.float32),
        "W1": rng.standard_normal((HD, 2 * D), dtype=np.float32) * (2 * D) ** -0.5,
        "b1": rng.standard_normal((HD,), dtype=np.float32) * (2 * D) ** -0.5,
        "W2": rng.standard_normal((1, HD), dtype=np.float32) * HD**-0.5,
        "b2": rng.standard_normal((1,), dtype=np.float32) * HD**-0.5,
    }
    o = kernel(**ins)
    print("kernel out:", o.shape, o.dtype, float(np.abs(o).max()))


# revision 55
# speedup vs baseline: 1.0021x; 1.0021x over previous
"""AffinityFC Trainium2 kernel (Bass/Tile, 8 NeuronCores, data-parallel over B).

Math per batch b (one NeuronCore per batch):
    px = X[b] @ W1x.T          (Nx=128, hd=1024)
    py = Y[b] @ W1y.T          (Ny=128, hd=1024)
    out[n, m] = W2 . relu(px[n, :] + py[m, :] + b1) + b2

Key reformulation: with s = px + b1,
    relu(py + s) = max(py, -s) + s
so for "max-form" rows the device computes u = max(py, -s) (one DVE
tensor_tensor max per element) and reduces Sum_h W2[h]*u with TensorE;
the missing Sum_h W2[h]*s[n,h] term is a per-chunk rank-1 correction
added on the host.  "relu-form" rows (ScalarE relu(py + s_n), bias per
partition) need no correction for their chunks.

Schedule (v20, ~78.8us fast-clock):
  - PSUM: obanks 0..5 allocated BEFORE the layer-1 pool (disjoint stack
    space, so main matmuls for those banks start immediately); obanks
    6,7 allocated in a sibling pool after layer-1's pool closes and
    reuse its 2 banks.  Bank map: nb 0..23 -> bank nb%6, nb 24..27 ->
    bank 7 (w3 DVE tile + ACT nb 27), nb 28..31 -> bank 6 (ACT rows).
  - DVE does only max ops (one w27 per middle chunk, w28 in chunk 0 —
    the DVE is the saturated engine, zero-gap back-to-back); ScalarE
    does the relu-form rows (relu ops with bias=s[:,n] per partition;
    rows 28-31 in chunks 0..6 and row 27 in chunks 1..6) plus all
    layer-1 evacs (negs/s/pyr) at high priority.
  - input DMAs: chunk-0-critical data merged into ONE DMA per HWDGE
    queue (crx=[xt|w1x_c0] on scalar, cry=[yt|w1y_c0] on sync, y first
    since pyr gates the first MAX), then the tiny consts, then bulk W1
    behind them in FIFO order; the SWDGE queue is never used.
  - last chunk: bank-grouped DVE ops (w4, last bank split w2+w2)
    feeding a per-bank matmul -> ScalarE evac -> out-DMA pipeline.
  Known hazard: the DVE/PE clocks vary between runs (0.96 vs 0.80 GHz
  observed); compare designs only at equal clock (MAX w24 = 6.47us
  fast, 7.87us slow).
"""

import numpy as np
import ml_dtypes

import concourse.mybir as mybir
import concourse.tile as tile
from concourse import bacc
from concourse.bass import ts
from concourse.bass_utils import run_bass_kernel_spmd

B, NX, NY, D, HD = 8, 128, 128, 512, 1024
NCORES = 8
NCH = HD // 128      # 8 h-chunks
KT = D // 128        # 4 k-tiles for the layer-1 contraction
NBLK = NX // 4       # 32 n-blocks of 4 rows each
F32 = mybir.dt.float32
BF16 = mybir.dt.bfloat16

ACT_NBS = (27, 28, 29, 30, 31)  # relu-form rows (ScalarE) in middle chunks


def relu_form(c, nb):
    # which (chunk, n-block) pairs are produced relu-form on ScalarE:
    # rows 28..31 in chunks 0..6 (chunk 0's relu ops run right after the
    # layer-1 evacs drain, taking them off the saturated DVE), row 27 in
    # the middle chunks only.
    if nb >= 28:
        return c < NCH - 1
    if nb == 27:
        return 1 <= c <= NCH - 2
    return False


def bankmap(nb):  # nb -> (bank, jc)
    if nb < 24:
        return nb % 6, nb // 6
    if nb < 28:
        return 7, nb - 24
    return 6, nb - 28


def _build_nc(do_compile=True):
    nc = bacc.Bacc(
        "TRN2", target_bir_lowering=False, debug=False, num_devices=NCORES
    )

    # flat SBUF images: dram[p, col] == sbuf[p, col]
    # crx/cry bundle the chunk-0-critical data into ONE DMA each:
    #   crx = [xt image (KT*NX) | w1x chunk-0 slab (KT*128)]
    CW0 = KT * 128
    crx = nc.dram_tensor("crx", [128, KT * NX + CW0], BF16, kind="ExternalInput")
    cry = nc.dram_tensor("cry", [128, KT * NY + CW0], BF16, kind="ExternalInput")
    w1xt = nc.dram_tensor("w1xt", [128, (NCH - 1) * CW0], BF16, kind="ExternalInput")
    w1yt = nc.dram_tensor("w1yt", [128, (NCH - 1) * CW0], BF16, kind="ExternalInput")
    b1c = nc.dram_tensor("b1c", [128, 2 * NCH], F32, kind="ExternalInput")
    w2c = nc.dram_tensor("w2c", [128, NCH * 32], BF16, kind="ExternalInput")
    out = nc.dram_tensor("out", [1, NBLK * 512], F32, kind="ExternalOutput")

    with tile.TileContext(nc) as tc:
        with (
            tc.tile_pool(name="const", bufs=1) as cp,
            tc.tile_pool(name="tprod", bufs=9) as tp,
        ):
            crx_sb = cp.tile([128, KT * NX + CW0], BF16)
            cry_sb = cp.tile([128, KT * NY + CW0], BF16)
            xt_sb = crx_sb[:, : KT * NX]
            yt_sb = cry_sb[:, : KT * NY]
            W1GRP = ((1, 2), (2, 4), (4, 6), (6, NCH))
            w1x_g = [
                cp.tile([128, (hi - lo) * KT * 128], BF16, name=f"w1x{lo}")
                for lo, hi in W1GRP
            ]
            w1y_g = [
                cp.tile([128, (hi - lo) * KT * 128], BF16, name=f"w1y{lo}")
                for lo, hi in W1GRP
            ]

            def w1slab(g, c, k):  # lhsT slab for (chunk c, k-tile)
                if c == 0:
                    cr = crx_sb if g is w1x_g else cry_sb
                    off = KT * NX + k * 128
                    return cr[:, off : off + 128]
                for (lo, hi), tile_ in zip(W1GRP, g):
                    if lo <= c < hi:
                        off = ((c - lo) * KT + k) * 128
                        return tile_[:, off : off + 128]
                raise AssertionError
            b1_sb = cp.tile([128, 2 * NCH], F32)    # [+b1 | -b1] chunk columns
            w2_sb = cp.tile([128, NCH * 32], BF16)
            negs_sb = cp.tile([128, HD], BF16)
            s_sb = cp.tile([128, HD], F32)
            pyr_sb = cp.tile([128, NCH * 512], BF16)  # py_rep4 per chunk
            out_sc = cp.tile([128, 8 * 512], F32)

            CW = KT * 128  # image columns per chunk
            # critical pieces on the 2 HWDGE queues, bulk W1 on SWDGE;
            # tiny consts lead (small-DMA completion latency ~1.3us)
            # one merged critical DMA per HWDGE queue; consts on SWDGE;
            # bulk W1 split behind the critical pieces (w1xt/w1yt carry
            # chunks 1..7, so group (lo,hi) maps to cols (lo-1)*CW0..)
            nc.sync.dma_start(out=cry_sb[:, :], in_=cry[:, :])
            nc.scalar.dma_start(out=crx_sb[:, :], in_=crx[:, :])
            # consts ride the HWDGE queues right behind the critical
            # bundles (needed ~1.5us after they land); SWDGE never used
            nc.sync.dma_start(out=b1_sb[:, :], in_=b1c[:, :])
            nc.scalar.dma_start(out=w2_sb[:, :], in_=w2c[:, :])
            for gi, (lo, hi) in enumerate(W1GRP):
                a, b = (lo - 1) * CW0, (hi - 1) * CW0
                nc.scalar.dma_start(out=w1x_g[gi][:, :], in_=w1xt[:, a:b])
                nc.sync.dma_start(out=w1y_g[gi][:, :], in_=w1yt[:, a:b])

            with tc.tile_pool(name="mpsA", bufs=1, space="PSUM") as mpsA:
                obanks = [None] * 8
                for i in range(6):
                    obanks[i] = mpsA.tile([128, 512], F32, name=f"ob{i}", tag=f"ob{i}")

                # ---- layer 1 per h-chunk; all evacs on ScalarE (hi-pri):
                #   negs = -(px+b1) bf16, s = px+b1 f32, pyr = py rep4 bf16
                with tc.tile_pool(name="l1ps", bufs=2, space="PSUM") as l1ps:
                    for c in range(NCH):
                        pyp = l1ps.tile([128, NY], F32, tag="l1")
                        for k in range(KT):
                            nc.tensor.matmul(
                                pyp[:, :],
                                w1slab(w1y_g, c, k),
                                yt_sb[:, ts(k, NY)],
                                start=(k == 0),
                                stop=(k == KT - 1),
                            )
                        with tc.high_priority():
                            nc.scalar.activation(
                                out=pyr_sb[:, ts(c, 512)].rearrange(
                                    "p (m j) -> p m j", j=4
                                ),
                                in_=pyp[:, :].unsqueeze(2).broadcast_to(
                                    (128, 128, 4)
                                ),
                                func=mybir.ActivationFunctionType.Copy,
                            )
                        pxp = l1ps.tile([128, NX], F32, tag="l1")
                        for k in range(KT):
                            nc.tensor.matmul(
                                pxp[:, :],
                                w1slab(w1x_g, c, k),
                                xt_sb[:, ts(k, NX)],
                                start=(k == 0),
                                stop=(k == KT - 1),
                            )
                        if c == 0:
                            # DVE is idle during the ramp and the ACT chain
                            # (pyr then negs) gates the first MAX otherwise
                            with tc.high_priority():
                                nc.vector.tensor_scalar(
                                    out=negs_sb[:, ts(c, 128)],
                                    in0=pxp[:, :],
                                    scalar1=b1_sb[:, c : c + 1],
                                    scalar2=-1.0,
                                    op0=mybir.AluOpType.add,
                                    op1=mybir.AluOpType.mult,
                                )
                        else:
                            with tc.high_priority():
                                nc.scalar.activation(
                                    out=negs_sb[:, ts(c, 128)],
                                    in_=pxp[:, :],
                                    func=mybir.ActivationFunctionType.Identity,
                                    bias=b1_sb[:, NCH + c : NCH + c + 1],
                                    scale=-1.0,
                                )
                        if c != NCH - 1:
                            with tc.high_priority():
                                nc.scalar.activation(
                                    out=s_sb[:, ts(c, 128)],
                                    in_=pxp[:, :],
                                    func=mybir.ActivationFunctionType.Identity,
                                    bias=b1_sb[:, c : c + 1],
                                    scale=1.0,
                                )

                # banks 6,7 reuse layer-1's psum space (deps via allocator)
                mpsB_cm = tc.tile_pool(name="mpsB", bufs=1, space="PSUM")
                mpsB = mpsB_cm.__enter__()
                obanks[6] = mpsB.tile([128, 512], F32, name="ob6", tag="ob6")
                obanks[7] = mpsB.tile([128, 512], F32, name="ob7", tag="ob7")

                # ---- main loop, c-outer; slivers accumulate across chunks
                for c in range(NCH):
                    last = c == NCH - 1
                    pyr_c = pyr_sb[:, ts(c, 512)]
                    pyr3 = pyr_c.rearrange("p (m j) -> p m j", j=4)
                    tslice = {}  # nb -> (tile, column offset index)

                    def dve_max(t, w, in1, prio):
                        in0 = pyr3.unsqueeze(1).broadcast_to((128, w, 128, 4))
                        out_ap = t[:, :].rearrange(
                            "p (nbs m j) -> p nbs m j", nbs=w, m=128
                        )
                        if prio:
                            with tc.high_priority():
                                nc.vector.tensor_tensor(
                                    out=out_ap, in0=in0, in1=in1,
                                    op=mybir.AluOpType.max,
                                )
                        else:
                            nc.vector.tensor_tensor(
                                out=out_ap, in0=in0, in1=in1,
                                op=mybir.AluOpType.max,
                            )

                    def negs_in1(cols, w):
                        # cols: list-slice of negs columns [p, w, 4] -> bcast m
                        return cols.unsqueeze(2).broadcast_to((128, w, 128, 4))

                    if last:
                        # bank-grouped w4 ops so the final bank-major
                        # matmul+evac+DMA pipeline starts per-bank
                        nrr = negs_sb[:, ts(c, 128)].rearrange(
                            "p (nb j) -> p nb j", j=4
                        )
                        for bk in range(8):
                            nbs_list = [nb for nb in range(NBLK) if bankmap(nb)[0] == bk]
                            lo = nbs_list[0]
                            step = nbs_list[1] - nbs_list[0]
                            if bk < 7:
                                t = tp.tile(
                                    [128, 4 * 512], BF16, name=f"tb{bk}",
                                    tag="t4",
                                )
                                in1 = negs_in1(
                                    nrr[:, lo : lo + 3 * step + 1 : step, :], 4
                                )
                                dve_max(t, 4, in1, False)
                                for i, nb in enumerate(nbs_list):
                                    tslice[nb] = (t, i)
                            else:
                                for half in range(2):
                                    t = tp.tile(
                                        [128, 2 * 512], BF16,
                                        name=f"tb{bk}_{half}", tag="t2", bufs=2,
                                    )
                                    sub = nbs_list[2 * half : 2 * half + 2]
                                    in1 = negs_in1(
                                        nrr[:, sub[0] : sub[1] + 1 : step, :], 2
                                    )
                                    dve_max(t, 2, in1, False)
                                    for i, nb in enumerate(sub):
                                        tslice[nb] = (t, i)
                    elif c == 0:
                        t = tp.tile([128, 28 * 512], BF16, name="t0",
                                    tag="t", bufs=4)
                        in1 = negs_in1(
                            negs_sb[:, c * 128 : c * 128 + 112]
                            .rearrange("p (nbs j) -> p nbs j", j=4), 28
                        )
                        dve_max(t, 28, in1, True)
                        for nbs in range(28):
                            tslice[nbs] = (t, nbs)
                        for nb in (28, 29, 30, 31):
                            ta = tp.tile(
                                [128, 512], BF16, name=f"ta{c}_{nb}", tag="ta",
                                bufs=6,
                            )
                            for j in range(4):
                                n = nb * 4 + j
                                nc.scalar.activation(
                                    out=ta[:, :].rearrange(
                                        "p (m j) -> p m j", j=4
                                    )[:, :, j],
                                    in_=pyr3[:, :, j],
                                    func=mybir.ActivationFunctionType.Relu,
                                    bias=s_sb[:, c * 128 + n : c * 128 + n + 1],
                                    scale=1.0,
                                )
                            tslice[nb] = (ta, 0)
                    else:
                        t = tp.tile([128, 27 * 512], BF16, name=f"t{c}",
                                    tag="t", bufs=4)
                        in1 = negs_in1(
                            negs_sb[:, c * 128 : c * 128 + 108]
                            .rearrange("p (nbs j) -> p nbs j", j=4), 27
                        )
                        dve_max(t, 27, in1, False)
                        for nbs in range(27):
                            tslice[nbs] = (t, nbs)
                        # ACT share (relu-form rows, bank 6)
                        for nb in ACT_NBS:
                            ta = tp.tile(
                                [128, 512], BF16, name=f"ta{c}_{nb}", tag="ta", bufs=6
                            )
                            for j in range(4):
                                n = nb * 4 + j
                                nc.scalar.activation(
                                    out=ta[:, :].rearrange("p (m j) -> p m j", j=4)[
                                        :, :, j
                                    ],
                                    in_=pyr3[:, :, j],
                                    func=mybir.ActivationFunctionType.Relu,
                                    bias=s_sb[:, c * 128 + n : c * 128 + n + 1],
                                    scale=1.0,
                                )
                            tslice[nb] = (ta, 0)

                    if not last:
                        for nb in range(NBLK):
                            bk, jc = bankmap(nb)
                            t, nbs = tslice[nb]
                            nc.tensor.matmul(
                                obanks[bk][32 * jc : 32 * jc + 32, :],
                                w2_sb[:, ts(c, 32)],
                                t[:, ts(nbs, 512)],
                                start=(c == 0),
                                stop=False,
                                tile_position=(0, 32 * jc),
                                skip_group_check=True,
                            )
                    else:
                        # bank-major: 4 slivers -> ScalarE evac -> out-DMA
                        for bk in range(8):
                            for nb in range(NBLK):
                                b2_, jc = bankmap(nb)
                                if b2_ != bk:
                                    continue
                                t, nbs = tslice[nb]
                                nc.tensor.matmul(
                                    obanks[bk][32 * jc : 32 * jc + 32, :],
                                    w2_sb[:, ts(c, 32)],
                                    t[:, ts(nbs, 512)],
                                    start=False,
                                    stop=True,
                                    tile_position=(0, 32 * jc),
                                    skip_group_check=True,
                                )
                            nc.scalar.copy(
                                out=out_sc[:, ts(bk, 512)], in_=obanks[bk][:, :]
                            )
                            # raw layout: raw[nb*512 + m*4 + j] for the 4 nb
                            # of this bank (jc = 0..3 at partitions 0/32/64/96)
                            nbs_list = [nb for nb in range(NBLK) if bankmap(nb)[0] == bk]
                            lo = nbs_list[0]
                            step = nbs_list[1] - nbs_list[0]
                            dst = out[:, :].rearrange(
                                "o (nb q) -> (o nb) q", nb=NBLK
                            )[lo : lo + 3 * step + 1 : step, :]
                            src = out_sc[0:128:32, ts(bk, 512)]
                            (nc.sync, nc.scalar)[bk % 2].dma_start(out=dst, in_=src)
                mpsB_cm.__exit__(None, None, None)

    if do_compile:
        nc.compile()
    return nc


_NC_CACHE = None


def _get_nc():
    global _NC_CACHE
    if _NC_CACHE is None:
        _NC_CACHE = _build_nc()
    return _NC_CACHE


def prepare_in_maps(X, Y, W1, b1, W2):
    X = np.asarray(X, dtype=np.float32)
    Y = np.asarray(Y, dtype=np.float32)
    W1 = np.asarray(W1, dtype=np.float32)
    b1 = np.asarray(b1, dtype=np.float32)
    W2 = np.asarray(W2, dtype=np.float32)

    bf = ml_dtypes.bfloat16

    def w1_img(Wh):  # (HD, D) -> flat sbuf image (128, KT*HD)
        return np.ascontiguousarray(
            Wh.reshape(NCH, 128, KT, 128).transpose(3, 0, 2, 1).reshape(128, -1)
        ).astype(bf)

    def xy_img(Xb):  # (N, D) -> flat sbuf image (128, KT*N)
        return np.ascontiguousarray(
            Xb.T.reshape(KT, 128, -1).transpose(1, 0, 2).reshape(128, -1)
        ).astype(bf)

    w1xi = w1_img(W1[:, :D])
    w1yi = w1_img(W1[:, D:])
    CW0 = KT * 128
    b1m = b1.reshape(NCH, 128).T                      # (128, NCH)
    b1cm = np.ascontiguousarray(np.hstack([b1m, -b1m]))  # [+b1 | -b1] f32
    w2cm = np.ascontiguousarray(
        np.repeat(W2.reshape(NCH, 128).T[:, :, None], 32, axis=2).reshape(128, -1)
    ).astype(bf)

    in_maps = []
    for b in range(B):
        in_maps.append(
            {
                "crx": np.ascontiguousarray(
                    np.hstack([xy_img(X[b]), w1xi[:, :CW0]])
                ),
                "cry": np.ascontiguousarray(
                    np.hstack([xy_img(Y[b]), w1yi[:, :CW0]])
                ),
                "w1xt": np.ascontiguousarray(w1xi[:, CW0:]),
                "w1yt": np.ascontiguousarray(w1yi[:, CW0:]),
                "b1c": b1cm,
                "w2c": w2cm,
            }
        )
    return in_maps


def postprocess(raw_outs, X, W1, b1, W2, b2):
    """raw[nb*512 + m*4 + j] = device sum for out row 4nb+j, col m.
    Add the per-(row, chunk-set) max-form correction gamma, then b2."""
    X = np.asarray(X, dtype=np.float32)
    W1 = np.asarray(W1, dtype=np.float32)
    b1 = np.asarray(b1, dtype=np.float32)
    W2 = np.asarray(W2, dtype=np.float32)
    b2 = np.asarray(b2, dtype=np.float32)

    # per-chunk rank-1 pieces: gam_c[b, n] = X[b, n]·v_c + g_c
    Vc = np.stack([
        W2[0, c * 128 : (c + 1) * 128] @ W1[c * 128 : (c + 1) * 128, :D]
        for c in range(NCH)
    ])  # (NCH, D)
    gc = np.array([
        W2[0, c * 128 : (c + 1) * 128] @ b1[c * 128 : (c + 1) * 128]
        for c in range(NCH)
    ])
    maxform = np.ones((NBLK, NCH), dtype=np.float32)
    for nb in range(NBLK):
        for c in range(NCH):
            if relu_form(c, nb):
                maxform[nb, c] = 0.0

    out = np.empty((B, NX, NY), dtype=np.float32)
    for b in range(B):
        r = raw_outs[b].reshape(NBLK, 128, 4)     # (nb, m, j)
        o = r.transpose(0, 2, 1).reshape(NX, NY)  # (4nb+j, m)
        A = X[b] @ Vc.T + gc                      # (NX, NCH)
        gamma = (A.reshape(NBLK, 4, NCH) * maxform[:, None, :]).sum(-1)
        out[b] = o + gamma.reshape(NX)[:, None] + b2[0]
    return out


def kernel(X, Y, W1, b1, W2, b2):
    in_maps = prepare_in_maps(X, Y, W1, b1, W2)
    nc = _get_nc()
    res = run_bass_kernel_spmd(nc, in_maps, core_ids=list(range(NCORES)))
    raw = [res.results[b]["out"].reshape(-1) for b in range(B)]
    return postprocess(raw, X, W1, b1, W2, b2)


if __name__ == "__main__":
    rng = np.random.default_rng(0)
    ins = {
        "X": rng.standard_normal((B, NX, D), dtype=np.float32),
        "Y": rng.standard_normal((B, NY, D), dtype=np.float32),
        "W1": rng.standard_normal((HD, 2 * D), dtype=np.float32) * (2 * D) ** -0.5,
        "b1": rng.standard_normal((HD,), dtype=np.float32) * (2 * D) ** -0.5,
        "W2": rng.standard_normal((1, HD), dtype=np.float32) * HD**-0.5,
        "b2": rng.standard_normal((1,), dtype=np.float32) * HD**-0.5,
    }
    o = kernel(**ins)
    print("kernel out:", o.shape, o.dtype, float(np.abs(o).max()))
